# revision 76
# baseline (speedup 1.0000x reference)
"""DiffHead (differential attention head) Trainium2 Bass kernel.

Strategy (hardcoded for B=8, T=2048, C=1024, HS=128, 8 cores):
  - Data-parallel over batch: one batch element per NeuronCore.
  - Host side only reshapes/shards: per-core q/k/v slices are passed
    transposed ([C, T]) so the projection matmuls contract over C on the
    partition axis. All FLOPs run on device. The first K-weight tile and
    the first score chunk ship in one packed head DMA so the PE starts
    ~3us into the program.
  - Scores are computed transposed (S^T[k, q]) in a trimmed causal band:
    k-tile j only covers queries q >= 128j. Masked fills of 1e-9 scale
    to exactly 1.0f after exp, so everything outside the band is
    closed-form (suffix sums of V rows + a masked count, built with one
    [16,16] suffix-mask matmul and broadcast-seeded into the stage-B
    accumulators via a DRAM-roundtripped seed row). The superdiagonal
    k-tile contributes exactly ONE real element per query tile
    (q=128i+127, k=128(i+1)); it is computed by a tiny gather chain and
    injected as a single-nonzero weights tile.
  - Stage B runs in NATURAL layout: each 128x128 u-band slice is the
    stationary operand and the moving operand is [V_j | ones-column],
    so the softmax denominator falls out of the same matmul as U@V
    (column 128 of the accumulator, both diff branches packed into one
    PSUM bank). No separate denominator matmuls, no reciprocal-row
    broadcasts: the per-query reciprocal is a [128,1] per-partition
    scalar in the natural combine, and the output leaves in natural
    [T, HS] layout.
  - Pad rows: r is zeroed for pad rows and (1-lambda)*mean(V) (an
    all-ones matmul over V) is added back, matching softmax-of-constant
    rows exactly.
  - Engine placement: diag-tile masks + correction-buffer memsets on the
    otherwise idle GpSimd/Pool engine; projection PSUM drains alternate
    ACT/DVE behind per-1024-col chunk stops; stage A interleaves with V
    projection and stage B, pacing the PE against the ACT exp stream.
"""

import numpy as np
import ml_dtypes

try:
    import concourse.bacc as bacc
except ImportError:  # pragma: no cover
    import sys

    sys.path.insert(0, "/opt/trn_rl_repo")
    import concourse.bacc as bacc

import concourse.mybir as mybir
import concourse.tile as tile
from concourse.bass_utils import run_bass_kernel_spmd

F32 = mybir.dt.float32
F32R = mybir.dt.float32r
BF16 = mybir.dt.bfloat16
FP8 = mybir.dt.float8e4
DR = mybir.MatmulPerfMode.DoubleRow
EXP = mybir.ActivationFunctionType.Exp
MUL = mybir.AluOpType.mult
ADD = mybir.AluOpType.add

HS = 128
LAMBDA_INIT = 0.8
N_CORES = 8
# fp8 weight pre-scale: W*WS on host keeps values out of fp8's subnormal
# range; compensated via exp scale (/WS^2) and the combine (/WS via npadn
# and the tvnbc factor).
WS = 1.0
CORR = True  # superdiagonal correction chain (bisect flag)


def _r(ap):
    """View an f32 AP as float32r so the PE runs at full rate."""
    return ap.bitcast(F32R)


def _band_widths(T, nT):
    """Score-band column width per k-tile j: queries q >= 128*j (the
    superdiagonal k-tile is closed-form except one element, corrected
    separately)."""
    return [T - 128 * j for j in range(nT)]


def build_nc(T=2048, C=1024, repeat=1, phase1_only=False):
    """Build the per-core Bass program. Same NEFF on all 8 cores (SPMD).

    repeat > 1 wraps the body in a hardware loop (for wall-clock slope
    timing); results are identical since the body is idempotent.
    """
    import contextlib

    nT = T // 128
    nC = C // 128
    nCP = nC // 2  # contraction-tile PAIRS (fp8 DoubleRow)
    SCALE = float(HS) ** -0.5 / (WS * WS)  # scores carry WS^2 from Wq*Wk

    widths = _band_widths(T, nT)
    band_off = [0] * nT
    for j in range(1, nT):
        band_off[j] = band_off[j - 1] + widths[j - 1]
    band_cols = band_off[-1] + widths[-1]

    def q_lo(j):
        return 128 * j

    # weights blocked per 128-row contraction tile. K ships alone (needed
    # first); Q+V pack into cbW8; bf16 consts pack into cbR.
    KCOLS = nC * 2 * HS
    WQ0 = 0
    WV0 = WQ0 + nC * 2 * HS
    W8COLS = WV0 + nC * HS
    IDB0 = 0
    ONE0 = IDB0 + 128
    TRISB0 = ONE0 + 128
    TRIS1M0 = TRISB0 + 128
    SUF160 = TRIS1M0 + 128
    RCOLS = SUF160 + nT
    # packed f32 blob offsets: [lvec | tiled lq1 | tiled lq2]
    LV0 = 0
    LQ1B0 = LV0 + 4
    LQ2B0 = LQ1B0 + 128
    F32COLS = LQ2B0 + 128

    nc = bacc.Bacc("TRN2", target_bir_lowering=False, num_devices=N_CORES)

    qT = nc.dram_tensor("qT", [C, T], BF16, kind="ExternalInput")
    kT = nc.dram_tensor("kT", [C, T], BF16, kind="ExternalInput")
    vT = nc.dram_tensor("vT", [C, T], BF16, kind="ExternalInput")
    cbK = nc.dram_tensor("cbK", [128, KCOLS], BF16, kind="ExternalInput")
    # head blob: [Wk(ct0) | kT rows 0:128 cols 0:512] -> ONE lead-in DMA
    cbH = nc.dram_tensor("cbH", [128, 2 * HS + 512], BF16, kind="ExternalInput")
    cbW8 = nc.dram_tensor("cbW8", [128, W8COLS], BF16, kind="ExternalInput")
    cbR = nc.dram_tensor("cbR", [128, RCOLS], BF16, kind="ExternalInput")
    cbf32 = nc.dram_tensor("cbf32", [128, F32COLS], F32, kind="ExternalInput")
    padb = nc.dram_tensor("padb", [128, 2 * nT], BF16, kind="ExternalInput")
    # scratch for reshaping seed rows onto partition 0 (partition->free)
    sscr = nc.dram_tensor("sscr", [1, nT, 129], BF16, kind="Internal")
    # output in natural layout [T, HS]
    out = nc.dram_tensor("out", [T, HS], F32, kind="ExternalOutput")

    with tile.TileContext(nc) as tc:
        rep_cm = tc.For_i(0, repeat, 1) if repeat > 1 else contextlib.nullcontext()
        with (
            rep_cm,
            tc.tile_pool(name="consts", bufs=1) as consts,
            tc.tile_pool(name="persist", bufs=1) as persist,
        ):
            # ---- packed constants (K weights first: needed immediately) ----
            cbk_sb = consts.tile([128, KCOLS], BF16, tag="cbK")
            cbh_sb = consts.tile([128, 2 * HS + 512], BF16, tag="cbH")
            # single head DMA: ct0 K-weights + the first score chunk's kT
            nc.sync.dma_start(cbh_sb, cbH.ap())
            cbw8_sb = consts.tile([128, W8COLS], BF16, tag="cbW8")
            cbr_sb = consts.tile([128, RCOLS], BF16, tag="cbR")
            cf = consts.tile([128, F32COLS], F32, tag="cbf32")
            pad_sb = consts.tile([128, 2 * nT], BF16, tag="padb")

            def wk_w(ct, h):
                if ct == 0:
                    return cbh_sb[:, h * HS : (h + 1) * HS]
                return cbk_sb[:, ct * 2 * HS + h * HS : ct * 2 * HS + (h + 1) * HS]

            def wq_w(ct, h):
                return cbw8_sb[
                    :, WQ0 + ct * 2 * HS + h * HS : WQ0 + ct * 2 * HS + (h + 1) * HS
                ]

            def wv_w(ct):
                return cbw8_sb[:, WV0 + ct * HS : WV0 + (ct + 1) * HS]

            idb_w = cbr_sb[:, IDB0 : IDB0 + 128]
            ones_w = cbr_sb[:, ONE0 : ONE0 + 128]  # all-ones [128,128]
            onesr_bw = cbr_sb[0:1, ONE0 : ONE0 + 128]  # ones row [1,128]
            trisdb = cbr_sb[:, TRISB0 : TRISB0 + 128]
            trisd1m = cbr_sb[:, TRIS1M0 : TRIS1M0 + 128]
            # [16,16] suffix mask: suf16[j, i] = 1 iff j >= i+1
            suf16_w = cbr_sb[0:nT, SUF160 : SUF160 + nT]
            lq1b_w = cf[:, LQ1B0 : LQ1B0 + 128]
            lq2b_w = cf[:, LQ2B0 : LQ2B0 + 128]
            lv_sb = cf[:, LV0 : LV0 + 4]

            def padn_col(i):
                return pad_sb[:, i : i + 1]

            def npadn_col(i):
                return pad_sb[:, nT + i : nT + i + 1]

            # ---- persistent intermediates ----
            q1t = persist.tile([128, T], F32R, tag="q1t")
            q2t = persist.tile([128, T], F32R, tag="q2t")
            k1t = persist.tile([128, T], F32R, tag="k1t")
            k2t = persist.tile([128, T], F32R, tag="k2t")
            # V natural blocks with a ones column: vx[:, j, 0:128]=V_j,
            # vx[:, j, 128]=1 (denominator column of the stage-B matmul)
            vx = persist.tile([128, nT, 129], BF16, tag="vx")
            seedrow = persist.tile([1, nT, 129], BF16, tag="seed")
            tvnbc = persist.tile([128, HS], F32, tag="tvnbc")
            lamc = persist.tile([128, 1], F32, tag="lamc")
            neglam = persist.tile([128, 1], F32, tag="neglam")
            ostage = persist.tile([128, nT, HS], F32, tag="ostage")
            u_band = [
                persist.tile([128, band_cols], BF16, tag="u1", name="u1"),
                persist.tile([128, band_cols], BF16, tag="u2", name="u2"),
            ]
            # superdiagonal-correction weights: zero except [0, i, 127]
            ucbuf = [
                persist.tile([128, nT - 1, 128], BF16, tag=f"uc{b}",
                             name=f"uc{b}")
                for b in range(2)
            ]

            with tc.tile_pool(name="xs", bufs=6) as xs:
                # ============ K and Q projections (full-PSUM scope) ============
                # 1024-col PSUM chunk tiles: the next projection's first
                # matmul only waits on the first drained chunk, not on the
                # whole [128, T] accumulator pair.
                with tc.tile_pool(name="ppK", bufs=4, space="PSUM") as ppK:
                    for xi, (xdram, w_of, outs) in enumerate(
                        ((kT, wk_w, (k1t, k2t)), (qT, wq_w, (q1t, q2t)))
                    ):
                        ps = [
                            ppK.tile([128, 1024], F32, tag="proj", name=f"ps{h}{f}")
                            for h in range(2)
                            for f in range(2)
                        ]
                        for ct in range(nC):
                            xt = xs.tile([128, T], BF16, tag="xt")
                            if xi == 0 and ct == 0:
                                # cols 0:512 ride in the head blob; fetch the
                                # rest, then the bulky K-weight remainder
                                nc.sync.dma_start(
                                    xt[:, 512:T], xdram.ap()[0:128, 512:T]
                                )
                                # split the K-weight remainder so kt1 isn't
                                # stuck behind one bulky transfer
                                nc.sync.dma_start(
                                    cbk_sb[:, 2 * HS : 8 * HS],
                                    cbK.ap()[:, 2 * HS : 8 * HS],
                                )
                            else:
                                nc.sync.dma_start(
                                    xt, xdram.ap()[ct * 128 : (ct + 1) * 128, :]
                                )
                            if xi == 0 and ct == 1:
                                nc.sync.dma_start(
                                    cbk_sb[:, 8 * HS : KCOLS],
                                    cbK.ap()[:, 8 * HS : KCOLS],
                                )
                            if xi == 0 and ct == 7:
                                # consts on the SP queue, in need-by order
                                nc.sync.dma_start(cbw8_sb, cbW8.ap())
                                nc.sync.dma_start(cf, cbf32.ap())
                                nc.sync.dma_start(cbr_sb, cbR.ap())
                                nc.sync.dma_start(pad_sb, padb.ap())
                            for h in range(2):
                                for f in range(2):
                                    for n0 in range(0, 1024, 512):
                                        c0 = f * 1024 + n0
                                        xsrc = (
                                            cbh_sb[:, 2 * HS : 2 * HS + 512]
                                            if xi == 0 and ct == 0 and c0 == 0
                                            else xt[:, c0 : c0 + 512]
                                        )
                                        nc.tensor.matmul(
                                            ps[2 * h + f][:, n0 : n0 + 512],
                                            w_of(ct, h),
                                            xsrc,
                                            start=(ct == 0),
                                            stop=(ct == nC - 1),
                                        )
                                    if ct == nC - 1:
                                        # drain right behind each chunk's stop
                                        eng = (
                                            nc.scalar.copy
                                            if f == 0
                                            else nc.vector.tensor_copy
                                        )
                                        eng(
                                            outs[h][:, f * 1024 : (f + 1) * 1024],
                                            ps[2 * h + f],
                                        )

                # ============ phase 2 ============
                ATILE = 1024  # psA tile width (2 PSUM banks)
                apool = [None]  # current score-PSUM pool (swapped per phase)
                with (
                    tc.tile_pool(name="sb2", bufs=2) as sb2,
                    tc.tile_pool(name="obufs", bufs=4) as obufs,
                    tc.tile_pool(name="rbufs", bufs=8) as rbufs,
                ):
                    def stage_a(j, t0s=None):
                        """Scores -> exp into u bands -> mask, both branches."""
                        w = widths[j]
                        for br in range(2):
                            KT = (k1t, k2t)[br]
                            QT = (q1t, q2t)[br]
                            for t0 in (t0s if t0s is not None
                                       else range(0, w, ATILE)):
                                tw = min(ATILE, w - t0)
                                s_ps = apool[0].tile([128, ATILE], F32, tag="s")
                                for n0 in range(0, tw, 512):
                                    nw = min(512, tw - n0)
                                    nc.tensor.matmul(
                                        s_ps[:, n0 : n0 + nw],
                                        KT[:, j * 128 : (j + 1) * 128],
                                        QT[:, q_lo(j) + t0 + n0 : q_lo(j) + t0 + n0 + nw],
                                        start=True,
                                        stop=True,
                                    )
                                nc.scalar.activation(
                                    u_band[br][
                                        :, band_off[j] + t0 : band_off[j] + t0 + tw
                                    ],
                                    s_ps[:, :tw],
                                    EXP,
                                    scale=SCALE,
                                )
                                if t0 == 0:
                                    # diagonal-tile mask applied AFTER exp,
                                    # off the score->exp critical path:
                                    # u*m+(1-m) forces masked entries to 1
                                    useg = u_band[br][
                                        :, band_off[j] : band_off[j] + 128
                                    ]
                                    nc.gpsimd.tensor_mul(useg, useg, trisdb)
                                    nc.gpsimd.tensor_add(useg, useg, trisd1m)

                    ctxA1 = tc.tile_pool(name="psA1", bufs=2, space="PSUM")
                    psA1 = ctxA1.__enter__()
                    apool[0] = psA1

                    # ---- V projection interleaved with stage A j=0..4 ----
                    # two [128,1024] accumulators: the first aliases the
                    # earliest-drained projection banks, starting V sooner
                    with tc.tile_pool(name="ppV", bufs=1, space="PSUM") as ppV:
                        vps = [
                            ppV.tile([128, 1024], F32, tag=f"v{f}", name=f"v{f}")
                            for f in range(2)
                        ]
                        vtb = obufs.tile([128, T], BF16, tag="vtb", bufs=1)
                        for ct in range(nC):
                            xt = xs.tile([128, T], BF16, tag="xt")
                            nc.sync.dma_start(xt, vT.ap()[ct * 128 : (ct + 1) * 128, :])
                            for n0 in range(0, T, 512):
                                nc.tensor.matmul(
                                    vps[n0 // 1024][:, n0 % 1024 : n0 % 1024 + 512],
                                    wv_w(ct),
                                    xt[:, n0 : n0 + 512],
                                    start=(ct == 0),
                                    stop=(ct == nC - 1),
                                )
                            if ct < 5 and not phase1_only:
                                stage_a(ct)
                        for n0 in range(0, T, 1024):
                            nc.vector.tensor_copy(
                                vtb[:, n0 : n0 + 1024], vps[n0 // 1024]
                            )

                    # V natural blocks (vx) + ones column
                    with tc.tile_pool(name="ppT", bufs=4, space="PSUM") as ppT:
                        for j in range(nT):
                            vtr = ppT.tile([128, 128], BF16, tag="m")
                            nc.tensor.transpose(
                                vtr, vtb[:, j * 128 : (j + 1) * 128], idb_w
                            )
                            nc.vector.tensor_copy(vx[:, j, 0:128], vtr)
                    nc.vector.memset(vx[:, :, 128:129], 1.0)

                    # tvnbc[q, h] = (1-lambda) * mean(V)[h] (same every row):
                    # accumulate sum_j sum_k V[k, h] via all-ones weights, then
                    # scale by (1-lambda)/T.
                    # seed rows: seedrow[0, i, 0:128] = sum_{j>=i+2} colsum(V_j),
                    # seedrow[0, i, 128] = 128*max(0, nT-2-i) (masked count);
                    # both fall out of one [16,16] suffix-mask matmul over the
                    # per-tile V column sums (count col: X col 128 = 128.0).
                    # A DRAM roundtrip reshapes [16,129] onto partition 0.
                    with tc.tile_pool(name="ppS", bufs=1, space="PSUM") as ppS:
                        # lambda via host-tiled weights: dots_bc[p, i] =
                        # sum_d lqi[d]*lki[d] on every partition
                        dots_ps = ppS.tile([128, 2], F32, tag="lam")
                        nc.tensor.matmul(
                            dots_ps[:, 0:1], lq1b_w, lv_sb[:, 1:2],
                            start=True, stop=True,
                        )
                        nc.tensor.matmul(
                            dots_ps[:, 1:2], lq2b_w, lv_sb[:, 3:4],
                            start=True, stop=True,
                        )
                        eexp = consts.tile([128, 2], F32, tag="eexp")
                        nc.scalar.activation(eexp, dots_ps, EXP)
                        nc.vector.tensor_sub(lamc, eexp[:, 0:1], eexp[:, 1:2])
                        nc.vector.tensor_scalar_add(lamc, lamc, LAMBDA_INIT)
                        nc.vector.tensor_scalar_mul(neglam, lamc, -1.0)

                        vca = obufs.tile([128, nT], BF16, tag="vca", bufs=1)
                        with nc.allow_low_precision(
                            reason="bf16 V column sums feed small seed-row "
                            "corrections; well within error budget"
                        ):
                            nc.vector.tensor_reduce(
                                vca,
                                vtb.rearrange("p (j q) -> p j q", j=nT),
                                mybir.AxisListType.X,
                                mybir.AluOpType.add,
                            )
                        vct_ps = ppS.tile([nT, 128], BF16, tag="vct")
                        nc.tensor.transpose(vct_ps, vca, idb_w)
                        vctx = obufs.tile([nT, 129], BF16, tag="vctx", bufs=1)
                        nc.vector.tensor_copy(vctx[:, 0:128], vct_ps)
                        nc.vector.memset(vctx[:, 128:129], 128.0)
                        seed_ps = ppS.tile([nT, 129], F32, tag="sd")
                        nc.tensor.matmul(
                            seed_ps, suf16_w, vctx, start=True, stop=True
                        )
                        seed_sb = obufs.tile([nT, 129], BF16, tag="seedT", bufs=1)
                        nc.vector.tensor_copy(seed_sb, seed_ps)
                        nc.sync.dma_start(sscr.ap()[0], seed_sb)
                        nc.sync.dma_start(seedrow, sscr.ap())

                        tvps = ppS.tile([128, HS], F32, tag="tv")
                        for j in range(nT):
                            nc.tensor.matmul(
                                tvps, ones_w, vx[:, j, 0:128],
                                start=(j == 0), stop=(j == nT - 1),
                            )
                        fac = consts.tile([128, 1], F32, tag="fac")
                        nc.vector.tensor_scalar_mul(fac, lamc, -1.0 / (T * WS))
                        nc.vector.tensor_scalar_add(fac, fac, 1.0 / (T * WS))
                        nc.vector.tensor_scalar(tvnbc, tvps, fac, None, MUL)

                    # superdiagonal correction: the trimmed k-tile i+1 is
                    # closed-form EXCEPT s(128i+127, 128(i+1)). Gather
                    # those q/k columns, dot via matmul, take the diagonal
                    # (mask+ones-reduce), exp, minus 1 -> one value per i
                    # scattered into a zeroed weights buffer; stage B adds
                    # delta * [V_row(128(i+1)) | 1] via a tiny matmul.
                    with tc.tile_pool(name="ppC", bufs=1, space="PSUM") as ppC:
                        for br in range(2 if CORR else 0):
                            QT = (q1t, q2t)[br]
                            KT = (k1t, k2t)[br]
                            Qg = QT.bitcast(F32).rearrange(
                                "p (i r) -> p i r", r=128
                            )[:, 0 : nT - 1, 127]
                            Kg = KT.bitcast(F32).rearrange(
                                "p (i r) -> p i r", r=128
                            )[:, 1:nT, 0]
                            prod = obufs.tile([128, nT - 1], BF16, tag="gsb",
                                              bufs=2)
                            with nc.allow_low_precision(
                                reason="bf16 partial products of one score "
                                "element; feeds a tiny (u-1) correction"
                            ):
                                nc.vector.tensor_mul(prod, Qg, Kg)
                            grow = ppC.tile([1, nT - 1], F32, tag="grow")
                            nc.tensor.matmul(
                                grow, ones_w[:, 0:1], prod,
                                start=True, stop=True,
                            )
                            ucrow = obufs.tile([1, nT - 1], BF16, tag="ucr",
                                               bufs=2)
                            nc.scalar.activation(ucrow, grow, EXP, scale=SCALE)
                            with nc.allow_low_precision(
                                reason="bf16 (u-1) delta of a single band "
                                "element; absolute error ~1e-3 vs denominators "
                                "of ~T"
                            ):
                                nc.vector.tensor_scalar_add(ucrow, ucrow, -1.0)
                            nc.gpsimd.memset(ucbuf[br], 0.0)
                            for i in range(nT - 1):
                                nc.vector.tensor_copy(
                                    ucbuf[br][0:1, i, 127:128],
                                    ucrow[:, i : i + 1],
                                )

                    # ---- stage B: natural-layout accumulation ----
                    def _b_branch(i, pb, br):
                        for j in range(i + 1):
                            u_w = u_band[br][
                                :, band_off[j] + 128 * (i - j)
                                : band_off[j] + 128 * (i - j) + 128
                            ]
                            nc.tensor.matmul(
                                pb, u_w, vx[:, j, 0:129],
                                start=(j == 0), stop=False,
                                skip_group_check=True,
                            )
                        if CORR and i < nT - 1:
                            # superdiagonal single-element correction
                            nc.tensor.matmul(
                                pb, ucbuf[br][:, i, :], vx[:, i + 1, 0:129],
                                start=False, stop=False,
                                skip_group_check=True,
                            )
                        # broadcast seed row (suffix-V sums + masked count)
                        # last so B(0) never stalls on the seed roundtrip
                        nc.tensor.matmul(
                            pb, onesr_bw, seedrow[0:1, i, :],
                            start=False, stop=True, skip_group_check=True,
                        )

                    def stage_b(i, psN):
                        # both branches share one PSUM bank: [0:129 | 129:258]
                        ps = psN.tile([128, 258], F32, tag="bn")
                        _b_branch(i, ps[:, 0:129], 0)
                        _b_branch(i, ps[:, 129:258], 1)
                        return ps

                    def combine(i, ps):
                        ps2 = ps[:, 129:258]
                        r1 = rbufs.tile([128, 1], F32, tag="r1")
                        nc.vector.reciprocal(r1, ps[:, 128:129])
                        nc.vector.tensor_mul(r1, r1, npadn_col(i))
                        c1 = sb2.tile([128, HS], F32, tag="c1")
                        nc.vector.tensor_scalar(c1, ps[:, 0:128], r1, None, MUL)
                        r2 = rbufs.tile([128, 1], F32, tag="r2")
                        nc.vector.reciprocal(r2, ps2[:, 128:129])
                        nc.vector.scalar_tensor_tensor(
                            r2, r2, neglam, npadn_col(i), MUL, MUL
                        )
                        o = sb2.tile([128, HS], F32, tag="o")
                        nc.vector.scalar_tensor_tensor(
                            o, ps2[:, 0:128], r2, c1, MUL, ADD
                        )
                        nc.vector.scalar_tensor_tensor(
                            ostage[:, i, :], tvnbc, padn_col(i), o, MUL, ADD
                        )
                        if i == nT - 1:
                            nc.sync.dma_start(
                                out.ap()[i * 128 : (i + 1) * 128, :].rearrange(
                                    "(i p) h -> p i h", p=128
                                ),
                                ostage[:, i : i + 1, :],
                            )
                        elif i % 4 == 3:
                            g = i - 3
                            nc.sync.dma_start(
                                out.ap()[g * 128 : (g + 4) * 128, :].rearrange(
                                    "(i p) h -> p i h", p=128
                                ),
                                ostage[:, g : g + 4, :],
                            )
                        elif i == nT - 2:
                            g = 12
                            nc.sync.dma_start(
                                out.ap()[g * 128 : (g + 3) * 128, :].rearrange(
                                    "(i p) h -> p i h", p=128
                                ),
                                ostage[:, g : g + 3, :],
                            )

                    ctxA1.__exit__(None, None, None)

                    if not phase1_only:
                        with (
                            tc.tile_pool(name="psA2", bufs=2,
                                         space="PSUM") as psA2,
                            tc.tile_pool(name="psN", bufs=4,
                                         space="PSUM") as psN,
                        ):
                            apool[0] = psA2
                            # interleave remaining stage-A groups with stage-B
                            # tiles: B(i) needs band j=i, i.e. A(i) emitted.
                            stage_a(5)
                            stage_a(6)
                            for i in range(nT):
                                pss = stage_b(i, psN)  # single tile
                                if i + 7 < nT:
                                    stage_a(i + 7)
                                combine(i, pss)

    nc.compile()
    return nc


def _host_constants(T, nT):
    kl = np.arange(128)[:, None]
    ql = np.arange(128)[None, :]
    trid = (kl <= ql + 1).astype(np.float32)
    tris = ((kl + 128) <= (ql + 1)).astype(np.float32)
    # suf16[j, i] = 1 iff j >= i+1 (in rows 0..nT-1 of a [128, nT] block)
    suf16 = np.zeros((128, nT), dtype=np.float32)
    suf16[:nT] = (np.arange(nT)[:, None] >= np.arange(nT)[None, :] + 1)
    return trid, tris, suf16


_NC_CACHE = {}


def make_in_maps(q, k, v, pad_mask, Wq, Wk, Wv, lq1, lk1, lq2, lk2):
    """Per-core input dicts (host-side sharding + layout marshaling)."""
    B, T, C = q.shape
    nT, nC = T // 128, C // 128
    bf16 = ml_dtypes.bfloat16
    fp8 = ml_dtypes.float8_e4m3
    trid, tris, suf16 = _host_constants(T, nT)

    def blocked(W):
        # [C, D] -> [128, nC*D] with cols ct*D + d = W[ct*128 + p, d]
        D = W.shape[1]
        return (
            np.asarray(W)
            .reshape(nC, 128, D)
            .transpose(1, 0, 2)
            .reshape(128, nC * D)
        )

    cbK = np.ascontiguousarray(blocked(Wk)).astype(bf16)
    cbW8 = np.ascontiguousarray(
        np.concatenate([blocked(Wq), blocked(Wv)], axis=1)
    ).astype(bf16)
    cbR = np.concatenate(
        [
            np.eye(128, dtype=np.float32),
            np.ones((128, 128), dtype=np.float32),
            trid,
            1.0 - trid,
            suf16,
        ],
        axis=1,
    ).astype(bf16)
    lvec = np.stack(
        [np.asarray(lq1), np.asarray(lk1), np.asarray(lq2), np.asarray(lk2)], axis=1
    ).astype(np.float32)
    lq1b = np.tile(np.asarray(lq1)[:, None], (1, 128))
    lq2b = np.tile(np.asarray(lq2)[:, None], (1, 128))
    cbf32 = np.concatenate([lvec, lq1b, lq2b], axis=1).astype(np.float32)

    in_maps = []
    for b in range(B):
        padf = np.asarray(pad_mask[b], dtype=np.float32).reshape(nT, 128).T
        # npadn carries the 1/WS numerator compensation (V is scaled by WS)
        padb = np.ascontiguousarray(
            np.concatenate([padf, (1.0 - padf) / WS], axis=1).astype(bf16)
        )
        kTb = np.ascontiguousarray(np.asarray(k[b]).T.astype(bf16))
        cbH = np.ascontiguousarray(
            np.concatenate([np.asarray(cbK[:, 0:256]), kTb[0:128, 0:512]], axis=1)
        )
        in_maps.append(
            dict(
                cbH=cbH,
                qT=np.ascontiguousarray(np.asarray(q[b]).T.astype(bf16)),
                kT=kTb,
                vT=np.ascontiguousarray(np.asarray(v[b]).T.astype(bf16)),
                cbK=cbK,
                cbW8=cbW8,
                cbR=cbR,
                cbf32=cbf32,
                padb=padb,
            )
        )
    return in_maps


def kernel(q, k, v, pad_mask, Wq, Wk, Wv, lq1, lk1, lq2, lk2):
    B, T, C = q.shape
    assert B == N_CORES
    key = (T, C)
    if key not in _NC_CACHE:
        _NC_CACHE[key] = build_nc(T=T, C=C)
    nc = _NC_CACHE[key]
    in_maps = make_in_maps(q, k, v, pad_mask, Wq, Wk, Wv, lq1, lk1, lq2, lk2)
    res = run_bass_kernel_spmd(nc, in_maps, core_ids=list(range(N_CORES)))
    return np.stack([np.asarray(r["out"]) for r in res.results], axis=0)
